# revision 36
# baseline (speedup 1.0000x reference)
"""BEVFeatureExtractorV2 Trainium2 kernel.

Computes, for each ROI box, 5 sample points (center + 4 edge midpoints of the
rotated box) and bilinearly interpolates a [C,H,W] BEV feature map at those
points, producing [B, N, 5*C].

Sharding: 8 cores = 4 batches x 2 halves of the 512 rois. Each core receives
its batch's feature map (re-laid-out on host) and 256 rois.

Device strategy (per core):
  - Host pre-lays the feature map as table2[y*W+x] = [im[y,x,:], im[y+1,x,:]]
    (shape [H*W, 2C], fp16) so ONE indirect-DMA descriptor (2KB) fetches all
    4 bilinear neighbors of a point: entries e and e+1 give pixels
    (y0,x0),(y0+1,x0),(y0,x0+1),(y0+1,x0+1).
  - On device: compute the 5 points per roi (sin on ACT engine), directly
    in pixel space pre-shifted by -0.5 so the HW round-to-nearest f32->i32
    convert yields floor(); gather indices in int arithmetic; 4 bilinear
    weights (f32, cast once to fp16 for the DVE muls); then for each of 10
    tiles of 128 points: indirect-gather [128, 4C] fp16, weight chunk 0
    on DVE (broadcast-AP multiply) and chunks 1-3 on ACT (Copy with
    per-partition f32 scale; asplit=3 relieves DVE<->GPSIMD SBUF-port
    contention that slows SWDGE descriptor gen), fold with 2 adds into a
    per-t [128, 5C] buffer, one HWDGE store per t. Output fp16, host
    upcasts to f32. Measured ~29.2 us per core amortized (bench.py For_i
    slope; rel err ~1e-3). asplit sweep: 1 -> 35.5, 2 -> 30.5, 3 -> 29.2,
    4 -> 33.9 (ACT 40-op chain stops hiding under the gathers). f32 folds
    (f32fold=1, kills DVE 2-port mode entirely) also regress: 35.9.

  Perf notes from this session (see bench.py variants):
  - The kernel is LATENCY-bound, not HBM-bound: each gpsimd
    indirect_dma_start costs ~2.4 us end-to-end nearly independent of
    payload (SWDGE fixed + completion latency, strictly serialized on the
    single qPoolDynamic queue). 10 gathers = ~24.5 us floor; f32->fp16
    halved gather bytes but saved only ~3 us.
  - Multi-offset indirect DMA (one call, many offsets) does NOT gather:
    HW uses idx[p, 0] per partition and streams the dest line contiguously
    (probe.py). dma_gather (mlp ucode, int16 idx) compiles via an
    overlapping elem_step AP but faults at runtime under this
    tile/axon build (probe_h.py) - abandoned.
  - Retargeting InstDMACopy.queue to qPoolDynamic{1..3} with
    num_swdge_queues=4 does not parallelize the gathers (no effect).
  - Wide per-t DVE folds (one broadcast mul over [P,5,4,C]) run at
    1 elem/cyc (broadcast AP defeats 16-bit packing) and serialize against
    the gathers - fine-grained per-(t,k) ops overlap better (30.0 vs 41.4).
"""

import os
import numpy as np

import concourse.bass as bass
import concourse.bacc as bacc
import concourse.tile as tile
from concourse import mybir
from concourse.bass_utils import run_bass_kernel_spmd

F32 = mybir.dt.float32
I32 = mybir.dt.int32

B, N, C, H, W = 4, 512, 256, 256, 256
NCORES = 8
NR = N * B // NCORES          # rois per core = 256
NPT = 5                       # sample points per roi
P = 128                       # partitions
NT = NR // P                  # roi tiles per core = 2
PC_START = -51.2
INV_VOX = 2.5                 # 1 / (0.1 * 4)
BIAS = -PC_START * INV_VOX    # 128.0

_CACHED = {}


def build_program(repeat=1, loop_iters=None, bufs=(3, 4, 4, 3), gg=1,
                  variant="full", nswq=1, dt16=False, vb=2, qrr=1, asplit=3,
                  f32fold=0):
    """variant: full | nostore | gather | gatherNk (N KB per point) |
    compute (single gather, full compute+store) | v2 family (single big
    gather + 3 wide DVE ops + single store; vb = pool depth).
    dt16: table/gather/weighting/output in fp16 (coords stay f32)."""
    import contextlib
    if variant.startswith("v2"):
        bufs = (vb, vb, vb, vb)
    do_compute = variant in ("full", "nostore", "compute",
                             "v2", "v2act", "v2nostore", "v2compute")
    do_store = variant in ("full", "compute", "v2", "v2act")
    DT = mybir.dt.float16 if dt16 else F32
    FDT = F32 if f32fold else DT
    dsz = 2 if dt16 else 4
    gbytes = 4 * C * dsz
    if variant.startswith("gather") and variant != "gather":
        gbytes = int(variant[6:-1]) * 1024
    nc = bacc.Bacc("TRN2", target_bir_lowering=False, debug=False,
                   enable_asserts=False, num_swdge_queues=nswq)
    table = nc.dram_tensor("table2", [H * W, 2 * C], DT, kind="ExternalInput").ap()
    rois = nc.dram_tensor("rois", [NR, 7], F32, kind="ExternalInput").ap()
    out = nc.dram_tensor("out", [NR, NPT * C], FDT, kind="ExternalOutput").ap()

    NJ = NT * NPT  # 10 gather tiles
    with tile.TileContext(nc) as tc:
        with tc.tile_pool(name="coord", bufs=2) as cp, \
             tc.tile_pool(name="gather", bufs=bufs[0]) as gp, \
             tc.tile_pool(name="mul", bufs=bufs[1]) as mp, \
             tc.tile_pool(name="fold", bufs=bufs[2]) as sp, \
             tc.tile_pool(name="outp", bufs=bufs[3]) as op, \
             (tc.For_i(0, loop_iters, 1) if loop_iters
              else contextlib.nullcontext()):
          for _rep in range(repeat):
            # ---- load rois: [256,7] -> [128, (t d)] -------------------
            R = cp.tile([P, NT * 7], F32)
            R3 = R[:].rearrange("p (t d) -> p t d", t=NT)
            nc.sync.dma_start(R3, rois.rearrange("(t p) d -> p t d", t=NT))

            cx = R3[:, :, 0]
            cy = R3[:, :, 1]
            ry = R3[:, :, 6]

            # ---- trig (ACT) -----------------------------------------
            zero = cp.tile([P, 1], F32)
            halfpi = cp.tile([P, 1], F32)
            nc.vector.memset(zero[:], 0.0)
            nc.vector.memset(halfpi[:], float(np.pi / 2))
            trig = cp.tile([P, 2 * NT], F32)
            t3 = trig[:].rearrange("p (a t) -> p a t", a=2)
            sn, cs = t3[:, 0, :], t3[:, 1, :]
            nc.scalar.activation(sn, ry, mybir.ActivationFunctionType.Sin,
                                 bias=zero[:])
            # cos(x) = sin(pi/2 - |x|), argument stays within [-pi/2, pi/2]
            ab = cp.tile([P, NT], F32)
            nc.scalar.activation(ab[:], ry, mybir.ActivationFunctionType.Abs,
                                 bias=zero[:])
            nc.scalar.activation(cs, ab[:], mybir.ActivationFunctionType.Sin,
                                 bias=halfpi[:], scale=-1.0)

            # ---- pixel-space center (ACT), pre-shifted by -0.5 ------
            # HW f32->i32 convert is round-to-nearest, so
            # convert(xs - 0.5) == floor(xs) (ties land on a value-correct
            # neighbor; frac is recomputed off the chosen neighbor below).
            ctr = cp.tile([P, 2 * NT], F32)
            c3 = ctr[:].rearrange("p (a t) -> p a t", a=2)
            xc, yc = c3[:, 0, :], c3[:, 1, :]
            nc.scalar.activation(xc, cx, mybir.ActivationFunctionType.Copy,
                                 bias=BIAS - 0.5, scale=INV_VOX)
            nc.scalar.activation(yc, cy, mybir.ActivationFunctionType.Copy,
                                 bias=BIAS - 0.5, scale=INV_VOX)

            # ---- scaled half-dims; 1.25 = 0.5*2.5 -------------------
            hd = cp.tile([P, 2 * NT], F32)
            h3 = hd[:].rearrange("p (a t) -> p a t", a=2)
            hx, hy = h3[:, 0, :], h3[:, 1, :]
            nc.vector.tensor_scalar_mul(hx, R3[:, :, 3], 1.25)
            nc.vector.tensor_scalar_mul(hy, R3[:, :, 4], 1.25)

            # ---- rotated pixel offsets (all DVE; Pool kept DMA-only)
            rot = cp.tile([P, 4 * NT], F32)
            r3 = rot[:].rearrange("p (a t) -> p a t", a=4)
            rxc, rxs, rys, ryc = (r3[:, a, :] for a in range(4))
            nc.vector.tensor_mul(rxc, hx, cs)
            nc.vector.tensor_mul(rys, hy, sn)
            nc.vector.tensor_mul(rxs, hx, sn)
            nc.vector.tensor_mul(ryc, hy, cs)

            # ---- 5 points per roi in pixel space --------------------
            XY = cp.tile([P, 2 * NJ], F32)
            x4 = XY[:].rearrange("p (a t k) -> p a t k", a=2, t=NT)
            xs3, ys3 = x4[:, 0, :, :], x4[:, 1, :, :]
            nc.vector.tensor_copy(xs3[:, :, 0], xc)
            nc.vector.tensor_sub(xs3[:, :, 1], xc, rxc)   # front
            nc.vector.tensor_add(xs3[:, :, 2], xc, rxc)   # back
            nc.vector.tensor_sub(xs3[:, :, 3], xc, rys)   # left
            nc.vector.tensor_add(xs3[:, :, 4], xc, rys)   # right
            nc.vector.tensor_copy(ys3[:, :, 0], yc)
            nc.vector.tensor_add(ys3[:, :, 1], yc, rxs)
            nc.vector.tensor_sub(ys3[:, :, 2], yc, rxs)
            nc.vector.tensor_sub(ys3[:, :, 3], yc, ryc)
            nc.vector.tensor_add(ys3[:, :, 4], yc, ryc)

            # ---- floor via RNE convert of pre-shifted coords --------
            XYi = cp.tile([P, 2 * NJ], I32)
            nc.vector.tensor_copy(XYi[:], XY[:])   # = floor(true coords)

            # ---- gather index in int arithmetic (gathers launch early)
            idx = cp.tile([P, NJ], I32)
            nc.vector.tensor_scalar(idx[:], XYi[:, NJ:], W, None,
                                    mybir.AluOpType.mult)
            nc.vector.tensor_add(idx[:], idx[:], XYi[:, :NJ])

            # ---- fracs + complements (overlap the gathers) ----------
            XYf = cp.tile([P, 2 * NJ], F32)
            D = cp.tile([P, 2 * NJ], F32)
            XYr = cp.tile([P, 2 * NJ], F32)
            XYg = cp.tile([P, 2 * NJ], F32)
            nc.vector.tensor_copy(XYf[:], XYi[:])
            nc.vector.tensor_sub(D[:], XY[:], XYf[:])        # frac - 0.5
            nc.vector.tensor_scalar(XYr[:], D[:], 0.5, None,
                                    mybir.AluOpType.add)     # frac
            nc.vector.tensor_scalar(XYg[:], D[:], -1.0, 0.5,
                                    mybir.AluOpType.mult, mybir.AluOpType.add)
            fx, fy = XYr[:, :NJ], XYr[:, NJ:]
            gx, gy = XYg[:, :NJ], XYg[:, NJ:]
            Wt = cp.tile([P, 4 * NJ], F32)
            W3 = Wt[:].rearrange("p (j w) -> p j w", w=4)
            nc.vector.tensor_mul(W3[:, :, 0], gx, gy)
            nc.vector.tensor_mul(W3[:, :, 1], gx, fy)
            nc.vector.tensor_mul(W3[:, :, 2], fx, gy)
            nc.vector.tensor_mul(W3[:, :, 3], fx, fy)
            if dt16:
                Wh = cp.tile([P, 4 * NJ], DT)
                nc.scalar.activation(Wh[:], Wt[:],
                                     mybir.ActivationFunctionType.Copy,
                                     bias=0.0, scale=1.0)
                W3h = Wh[:].rearrange("p (j w) -> p j w", w=4)
            else:
                W3h = W3
            # weights for the DVE mul must match the fold dtype
            W3f = W3 if (f32fold or not dt16) else W3h

            # ---- gather + weighted fold per (t, k) tile -------------
            # gg points gathered per indirect DMA (2D dest, flat layout)
            if variant.startswith("v2"):
                # ---- ONE indirect gather for all 10 point-tiles ------
                NJ4C = NJ * 4 * C
                Gb = gp.tile([P, NJ4C], DT, tag="G")
                if variant != "v2compute":
                    nc.gpsimd.indirect_dma_start(
                        out=Gb[:], out_offset=None, in_=table,
                        in_offset=bass.IndirectOffsetOnAxis(ap=idx[:], axis=0),
                    )
                if do_compute:
                    G4 = Gb[:].rearrange("p (j a c) -> p j a c", j=NJ, a=4)
                    Wb = (W3h[:, :, :].unsqueeze(3)
                          .to_broadcast([P, NJ, 4, C]))
                    M = mp.tile([P, NJ4C], DT, tag="M")
                    M4 = M[:].rearrange("p (j a c) -> p j a c", j=NJ, a=4)
                    if variant == "v2act":
                        # chunks 0-1 on DVE, chunks 2-3 on ACT per point
                        nc.vector.tensor_mul(
                            M4[:, :, 0:2, :], G4[:, :, 0:2, :],
                            W3h[:, :, 0:2].unsqueeze(3)
                            .to_broadcast([P, NJ, 2, C]))
                        for j in range(NJ):
                            nc.scalar.activation(
                                M4[:, j, 2, :], G4[:, j, 2, :],
                                mybir.ActivationFunctionType.Copy,
                                bias=0.0, scale=W3[:, j, 2:3])
                            nc.scalar.activation(
                                M4[:, j, 3, :], G4[:, j, 3, :],
                                mybir.ActivationFunctionType.Copy,
                                bias=0.0, scale=W3[:, j, 3:4])
                    else:
                        nc.vector.tensor_mul(M4, G4, Wb)
                    S = sp.tile([P, NJ * 2 * C], DT, tag="S")
                    S4 = S[:].rearrange("p (j a c) -> p j a c", j=NJ, a=2)
                    nc.vector.tensor_add(S4, M4[:, :, 0:2, :], M4[:, :, 2:4, :])
                    O = op.tile([P, NJ * C], DT, tag="O")
                    O4 = O[:].rearrange("p (t k c) -> p t k c", t=NT, k=NPT)
                    nc.vector.tensor_add(
                        O4, S4[:, :, 0, :].rearrange("p (t k) c -> p t k c",
                                                     t=NT),
                        S4[:, :, 1, :].rearrange("p (t k) c -> p t k c", t=NT))
                    if do_store:
                        out4 = out.rearrange("(t p) (k c) -> p t k c",
                                             t=NT, k=NPT)
                        nc.sync.dma_start(out4, O4)
                continue
            # ---- per-t: 5 gathers, then 3 wide DVE ops + 1 store -----
            gelem = gbytes // dsz
            Gfirst = None
            for t in range(NT):
                Gb = gp.tile([P, NPT * gelem], DT, tag="G")
                if variant == "compute" and Gfirst is not None:
                    Gb = Gfirst
                else:
                    for k in range(NPT):
                        j = t * NPT + k
                        nc.gpsimd.indirect_dma_start(
                            out=Gb[:, k * gelem:(k + 1) * gelem],
                            out_offset=None,
                            in_=table,
                            in_offset=bass.IndirectOffsetOnAxis(
                                ap=idx[:, j:j + 1], axis=0),
                        )
                    Gfirst = Gb
                if not do_compute:
                    continue
                O = op.tile([P, NPT * C], FDT, tag="O")
                for k in range(NPT):
                    j = t * NPT + k
                    G = Gb[:, k * 4 * C:(k + 1) * 4 * C]
                    # first (4-asplit) chunks weighted on DVE, rest on ACT
                    nd = 4 - asplit
                    M = mp.tile([P, 4 * C], FDT, tag="M")
                    if nd > 0:
                        nc.vector.tensor_mul(
                            M[:, :nd * C].rearrange("p (a c) -> p a c", a=nd),
                            G[:, :nd * C].rearrange("p (a c) -> p a c", a=nd),
                            W3f[:, j, 0:nd].unsqueeze(2)
                            .to_broadcast([P, nd, C]),
                        )
                    for a in range(nd, 4):
                        nc.scalar.activation(
                            M[:, a * C:(a + 1) * C], G[:, a * C:(a + 1) * C],
                            mybir.ActivationFunctionType.Copy,
                            bias=0.0, scale=W3[:, j, a:a + 1])
                    S = sp.tile([P, 2 * C], FDT, tag="S")
                    nc.vector.tensor_add(S[:], M[:, :2 * C], M[:, 2 * C:])
                    nc.vector.tensor_add(O[:, k * C:(k + 1) * C],
                                         S[:, :C], S[:, C:])
                if do_store:
                    nc.sync.dma_start(out[t * P:(t + 1) * P, :], O[:])
    nc.compile()
    return nc


def _get_program():
    if "nc" not in _CACHED:
        _CACHED["nc"] = build_program(dt16=True)
    return _CACHED["nc"]


def _make_table2(feats, dtype=np.float32):
    """feats: [B,C,H,W] f32 -> list of B arrays [H*W, 2C] (channel-last,
    row y and y+1 concatenated)."""
    tables = []
    for b in range(B):
        bev = np.ascontiguousarray(feats[b].transpose(1, 2, 0)).astype(dtype)
        nxt = bev[np.minimum(np.arange(H) + 1, H - 1)]           # [H,W,C]
        t2 = np.concatenate([bev, nxt], axis=2)                  # [H,W,2C]
        tables.append(np.ascontiguousarray(t2.reshape(H * W, 2 * C)))
    return tables


def kernel(spatial_features_2d, rois, _want_results=False):
    feats = np.asarray(spatial_features_2d, dtype=np.float32)
    rois_np = np.asarray(rois, dtype=np.float32)
    assert feats.shape == (B, C, H, W) and rois_np.shape == (B, N, 7)

    nc = _get_program()
    tables = _make_table2(feats, np.float16)
    in_maps = []
    for core in range(NCORES):
        b, h = divmod(core, 2)
        in_maps.append({
            "table2": tables[b],
            "rois": np.ascontiguousarray(rois_np[b, h * NR:(h + 1) * NR]),
        })

    try:
        res = run_bass_kernel_spmd(
            nc, in_maps, list(range(NCORES)),
            trace=bool(int(os.environ.get("BEV_TRACE", "0"))),
        )
    except ModuleNotFoundError:
        # BASS_TRACE in the environment routes through the NTFF profile
        # hook (antenv.axon_hooks), which some containers lack. Degrade to
        # an untraced run instead of failing.
        os.environ["BASS_NEVER_TRACE"] = "1"
        try:
            res = run_bass_kernel_spmd(nc, in_maps, list(range(NCORES)),
                                       trace=False)
        finally:
            os.environ.pop("BASS_NEVER_TRACE", None)

    out = np.empty((B, N, NPT * C), dtype=np.float32)
    for core in range(NCORES):
        b, h = divmod(core, 2)
        out[b, h * NR:(h + 1) * NR] = res.results[core]["out"].astype(
            np.float32)
    if _want_results:
        return out, res
    return out



# revision 39
# speedup vs baseline: 1.4407x; 1.4407x over previous
"""BEVFeatureExtractorV2 Trainium2 kernel.

Computes, for each ROI box, 5 sample points (center + 4 edge midpoints of the
rotated box) and bilinearly interpolates a [C,H,W] BEV feature map at those
points, producing [B, N, 5*C].

Sharding: 8 cores = 4 batches x 2 halves of the 512 rois. Each core receives
its batch's feature map (re-laid-out on host) and 256 rois.

Device strategy (per core):
  - Host pre-lays the feature map as table2[y*W+x] = [im[y,x,:], im[y+1,x,:]]
    (shape [H*W, 2C], fp16) so ONE indirect-DMA descriptor (2KB) fetches all
    4 bilinear neighbors of a point: entries e and e+1 give pixels
    (y0,x0),(y0+1,x0),(y0,x0+1),(y0+1,x0+1).
  - On device: compute the 5 points per roi (sin on ACT engine), directly
    in pixel space pre-shifted by -0.5 so the HW round-to-nearest f32->i32
    convert yields floor(); gather indices in int arithmetic; 4 bilinear
    weights (f32, cast once to fp16 for the DVE muls); then for each of 10
    tiles of 128 points: indirect-gather [128, 4C] fp16, weight chunk 0
    on DVE (broadcast-AP multiply) and chunks 1-3 on ACT (Copy with
    per-partition f32 scale; asplit=3 relieves DVE<->GPSIMD SBUF-port
    contention that slows SWDGE descriptor gen), fold with 2 adds into a
    per-t [128, 5C] buffer, one HWDGE store per t. Output fp16, host
    upcasts to f32. Measured ~29.2 us per core amortized (bench.py For_i
    slope; rel err ~1e-3). asplit sweep: 1 -> 35.5, 2 -> 30.5, 3 -> 29.2,
    4 -> 33.9 (ACT 40-op chain stops hiding under the gathers). f32 folds
    (f32fold=1, kills DVE 2-port mode entirely) also regress: 35.9.

  Perf notes from this session (see bench.py variants):
  - The kernel is LATENCY-bound, not HBM-bound: each gpsimd
    indirect_dma_start costs ~2.4 us end-to-end nearly independent of
    payload (SWDGE fixed + completion latency, strictly serialized on the
    single qPoolDynamic queue). 10 gathers = ~24.5 us floor; f32->fp16
    halved gather bytes but saved only ~3 us.
  - Multi-offset indirect DMA (one call, many offsets) does NOT gather:
    HW uses idx[p, 0] per partition and streams the dest line contiguously
    (probe.py). dma_gather (mlp ucode, int16 idx) compiles via an
    overlapping elem_step AP but faults at runtime under this
    tile/axon build (probe_h.py) - abandoned.
  - Retargeting InstDMACopy.queue to qPoolDynamic{1..3} with
    num_swdge_queues=4 does not parallelize the gathers (no effect).
  - Wide per-t DVE folds (one broadcast mul over [P,5,4,C]) run at
    1 elem/cyc (broadcast AP defeats 16-bit packing) and serialize against
    the gathers - fine-grained per-(t,k) ops overlap better (30.0 vs 41.4).
  - Unrolling MULTIPLE full passes per For_i body (repeat=2/4) is worth
    ~8 us/pass: per-pass 29.2 (repeat=1) -> 22.0 (2) -> 21.0 (4). The tile
    scheduler only interleaves within one body, so with repeat=1 the next
    pass's DVE coordinate chain queues behind this pass's folds and the
    gather queue starves between passes. Deeper pools (deep=1) regress.
"""

import os
import numpy as np

import concourse.bass as bass
import concourse.bacc as bacc
import concourse.tile as tile
from concourse import mybir
from concourse.bass_utils import run_bass_kernel_spmd

F32 = mybir.dt.float32
I32 = mybir.dt.int32

B, N, C, H, W = 4, 512, 256, 256, 256
NCORES = 8
NR = N * B // NCORES          # rois per core = 256
NPT = 5                       # sample points per roi
P = 128                       # partitions
NT = NR // P                  # roi tiles per core = 2
PC_START = -51.2
INV_VOX = 2.5                 # 1 / (0.1 * 4)
BIAS = -PC_START * INV_VOX    # 128.0

_CACHED = {}


def build_program(repeat=1, loop_iters=None, bufs=(3, 4, 4, 3), gg=1,
                  variant="full", nswq=1, dt16=False, vb=2, qrr=1, asplit=3,
                  f32fold=0, deep=0):
    """variant: full | nostore | gather | gatherNk (N KB per point) |
    compute (single gather, full compute+store) | v2 family (single big
    gather + 3 wide DVE ops + single store; vb = pool depth).
    dt16: table/gather/weighting/output in fp16 (coords stay f32)."""
    import contextlib
    if variant.startswith("v2"):
        bufs = (vb, vb, vb, vb)
    if deep:
        bufs = (5, 6, 6, 4)
    do_compute = variant in ("full", "nostore", "compute",
                             "v2", "v2act", "v2nostore", "v2compute")
    do_store = variant in ("full", "compute", "v2", "v2act")
    DT = mybir.dt.float16 if dt16 else F32
    FDT = F32 if f32fold else DT
    dsz = 2 if dt16 else 4
    gbytes = 4 * C * dsz
    if variant.startswith("gather") and variant != "gather":
        gbytes = int(variant[6:-1]) * 1024
    nc = bacc.Bacc("TRN2", target_bir_lowering=False, debug=False,
                   enable_asserts=False, num_swdge_queues=nswq)
    table = nc.dram_tensor("table2", [H * W, 2 * C], DT, kind="ExternalInput").ap()
    rois = nc.dram_tensor("rois", [NR, 7], F32, kind="ExternalInput").ap()
    out = nc.dram_tensor("out", [NR, NPT * C], FDT, kind="ExternalOutput").ap()

    NJ = NT * NPT  # 10 gather tiles
    with tile.TileContext(nc) as tc:
        with tc.tile_pool(name="coord", bufs=2) as cp, \
             tc.tile_pool(name="gather", bufs=bufs[0]) as gp, \
             tc.tile_pool(name="mul", bufs=bufs[1]) as mp, \
             tc.tile_pool(name="fold", bufs=bufs[2]) as sp, \
             tc.tile_pool(name="outp", bufs=bufs[3]) as op, \
             (tc.For_i(0, loop_iters, 1) if loop_iters
              else contextlib.nullcontext()):
          for _rep in range(repeat):
            # ---- load rois: [256,7] -> [128, (t d)] -------------------
            R = cp.tile([P, NT * 7], F32)
            R3 = R[:].rearrange("p (t d) -> p t d", t=NT)
            nc.sync.dma_start(R3, rois.rearrange("(t p) d -> p t d", t=NT))

            cx = R3[:, :, 0]
            cy = R3[:, :, 1]
            ry = R3[:, :, 6]

            # ---- trig (ACT) -----------------------------------------
            zero = cp.tile([P, 1], F32)
            halfpi = cp.tile([P, 1], F32)
            nc.vector.memset(zero[:], 0.0)
            nc.vector.memset(halfpi[:], float(np.pi / 2))
            trig = cp.tile([P, 2 * NT], F32)
            t3 = trig[:].rearrange("p (a t) -> p a t", a=2)
            sn, cs = t3[:, 0, :], t3[:, 1, :]
            nc.scalar.activation(sn, ry, mybir.ActivationFunctionType.Sin,
                                 bias=zero[:])
            # cos(x) = sin(pi/2 - |x|), argument stays within [-pi/2, pi/2]
            ab = cp.tile([P, NT], F32)
            nc.scalar.activation(ab[:], ry, mybir.ActivationFunctionType.Abs,
                                 bias=zero[:])
            nc.scalar.activation(cs, ab[:], mybir.ActivationFunctionType.Sin,
                                 bias=halfpi[:], scale=-1.0)

            # ---- pixel-space center (ACT), pre-shifted by -0.5 ------
            # HW f32->i32 convert is round-to-nearest, so
            # convert(xs - 0.5) == floor(xs) (ties land on a value-correct
            # neighbor; frac is recomputed off the chosen neighbor below).
            ctr = cp.tile([P, 2 * NT], F32)
            c3 = ctr[:].rearrange("p (a t) -> p a t", a=2)
            xc, yc = c3[:, 0, :], c3[:, 1, :]
            nc.scalar.activation(xc, cx, mybir.ActivationFunctionType.Copy,
                                 bias=BIAS - 0.5, scale=INV_VOX)
            nc.scalar.activation(yc, cy, mybir.ActivationFunctionType.Copy,
                                 bias=BIAS - 0.5, scale=INV_VOX)

            # ---- scaled half-dims; 1.25 = 0.5*2.5 -------------------
            hd = cp.tile([P, 2 * NT], F32)
            h3 = hd[:].rearrange("p (a t) -> p a t", a=2)
            hx, hy = h3[:, 0, :], h3[:, 1, :]
            nc.vector.tensor_scalar_mul(hx, R3[:, :, 3], 1.25)
            nc.vector.tensor_scalar_mul(hy, R3[:, :, 4], 1.25)

            # ---- rotated pixel offsets (all DVE; Pool kept DMA-only)
            rot = cp.tile([P, 4 * NT], F32)
            r3 = rot[:].rearrange("p (a t) -> p a t", a=4)
            rxc, rxs, rys, ryc = (r3[:, a, :] for a in range(4))
            nc.vector.tensor_mul(rxc, hx, cs)
            nc.vector.tensor_mul(rys, hy, sn)
            nc.vector.tensor_mul(rxs, hx, sn)
            nc.vector.tensor_mul(ryc, hy, cs)

            # ---- 5 points per roi in pixel space --------------------
            XY = cp.tile([P, 2 * NJ], F32)
            x4 = XY[:].rearrange("p (a t k) -> p a t k", a=2, t=NT)
            xs3, ys3 = x4[:, 0, :, :], x4[:, 1, :, :]
            nc.vector.tensor_copy(xs3[:, :, 0], xc)
            nc.vector.tensor_sub(xs3[:, :, 1], xc, rxc)   # front
            nc.vector.tensor_add(xs3[:, :, 2], xc, rxc)   # back
            nc.vector.tensor_sub(xs3[:, :, 3], xc, rys)   # left
            nc.vector.tensor_add(xs3[:, :, 4], xc, rys)   # right
            nc.vector.tensor_copy(ys3[:, :, 0], yc)
            nc.vector.tensor_add(ys3[:, :, 1], yc, rxs)
            nc.vector.tensor_sub(ys3[:, :, 2], yc, rxs)
            nc.vector.tensor_sub(ys3[:, :, 3], yc, ryc)
            nc.vector.tensor_add(ys3[:, :, 4], yc, ryc)

            # ---- floor via RNE convert of pre-shifted coords --------
            XYi = cp.tile([P, 2 * NJ], I32)
            nc.vector.tensor_copy(XYi[:], XY[:])   # = floor(true coords)

            # ---- gather index in int arithmetic (gathers launch early)
            idx = cp.tile([P, NJ], I32)
            nc.vector.tensor_scalar(idx[:], XYi[:, NJ:], W, None,
                                    mybir.AluOpType.mult)
            nc.vector.tensor_add(idx[:], idx[:], XYi[:, :NJ])

            # ---- fracs + complements (overlap the gathers) ----------
            XYf = cp.tile([P, 2 * NJ], F32)
            D = cp.tile([P, 2 * NJ], F32)
            XYr = cp.tile([P, 2 * NJ], F32)
            XYg = cp.tile([P, 2 * NJ], F32)
            nc.vector.tensor_copy(XYf[:], XYi[:])
            nc.vector.tensor_sub(D[:], XY[:], XYf[:])        # frac - 0.5
            nc.vector.tensor_scalar(XYr[:], D[:], 0.5, None,
                                    mybir.AluOpType.add)     # frac
            nc.vector.tensor_scalar(XYg[:], D[:], -1.0, 0.5,
                                    mybir.AluOpType.mult, mybir.AluOpType.add)
            fx, fy = XYr[:, :NJ], XYr[:, NJ:]
            gx, gy = XYg[:, :NJ], XYg[:, NJ:]
            Wt = cp.tile([P, 4 * NJ], F32)
            W3 = Wt[:].rearrange("p (j w) -> p j w", w=4)
            nc.vector.tensor_mul(W3[:, :, 0], gx, gy)
            nc.vector.tensor_mul(W3[:, :, 1], gx, fy)
            nc.vector.tensor_mul(W3[:, :, 2], fx, gy)
            nc.vector.tensor_mul(W3[:, :, 3], fx, fy)
            if dt16:
                Wh = cp.tile([P, 4 * NJ], DT)
                nc.scalar.activation(Wh[:], Wt[:],
                                     mybir.ActivationFunctionType.Copy,
                                     bias=0.0, scale=1.0)
                W3h = Wh[:].rearrange("p (j w) -> p j w", w=4)
            else:
                W3h = W3
            # weights for the DVE mul must match the fold dtype
            W3f = W3 if (f32fold or not dt16) else W3h

            # ---- gather + weighted fold per (t, k) tile -------------
            # gg points gathered per indirect DMA (2D dest, flat layout)
            if variant.startswith("v2"):
                # ---- ONE indirect gather for all 10 point-tiles ------
                NJ4C = NJ * 4 * C
                Gb = gp.tile([P, NJ4C], DT, tag="G")
                if variant != "v2compute":
                    nc.gpsimd.indirect_dma_start(
                        out=Gb[:], out_offset=None, in_=table,
                        in_offset=bass.IndirectOffsetOnAxis(ap=idx[:], axis=0),
                    )
                if do_compute:
                    G4 = Gb[:].rearrange("p (j a c) -> p j a c", j=NJ, a=4)
                    Wb = (W3h[:, :, :].unsqueeze(3)
                          .to_broadcast([P, NJ, 4, C]))
                    M = mp.tile([P, NJ4C], DT, tag="M")
                    M4 = M[:].rearrange("p (j a c) -> p j a c", j=NJ, a=4)
                    if variant == "v2act":
                        # chunks 0-1 on DVE, chunks 2-3 on ACT per point
                        nc.vector.tensor_mul(
                            M4[:, :, 0:2, :], G4[:, :, 0:2, :],
                            W3h[:, :, 0:2].unsqueeze(3)
                            .to_broadcast([P, NJ, 2, C]))
                        for j in range(NJ):
                            nc.scalar.activation(
                                M4[:, j, 2, :], G4[:, j, 2, :],
                                mybir.ActivationFunctionType.Copy,
                                bias=0.0, scale=W3[:, j, 2:3])
                            nc.scalar.activation(
                                M4[:, j, 3, :], G4[:, j, 3, :],
                                mybir.ActivationFunctionType.Copy,
                                bias=0.0, scale=W3[:, j, 3:4])
                    else:
                        nc.vector.tensor_mul(M4, G4, Wb)
                    S = sp.tile([P, NJ * 2 * C], DT, tag="S")
                    S4 = S[:].rearrange("p (j a c) -> p j a c", j=NJ, a=2)
                    nc.vector.tensor_add(S4, M4[:, :, 0:2, :], M4[:, :, 2:4, :])
                    O = op.tile([P, NJ * C], DT, tag="O")
                    O4 = O[:].rearrange("p (t k c) -> p t k c", t=NT, k=NPT)
                    nc.vector.tensor_add(
                        O4, S4[:, :, 0, :].rearrange("p (t k) c -> p t k c",
                                                     t=NT),
                        S4[:, :, 1, :].rearrange("p (t k) c -> p t k c", t=NT))
                    if do_store:
                        out4 = out.rearrange("(t p) (k c) -> p t k c",
                                             t=NT, k=NPT)
                        nc.sync.dma_start(out4, O4)
                continue
            # ---- per-t: 5 gathers, then 3 wide DVE ops + 1 store -----
            gelem = gbytes // dsz
            Gfirst = None
            for t in range(NT):
                Gb = gp.tile([P, NPT * gelem], DT, tag="G")
                if variant == "compute" and Gfirst is not None:
                    Gb = Gfirst
                else:
                    for k in range(NPT):
                        j = t * NPT + k
                        nc.gpsimd.indirect_dma_start(
                            out=Gb[:, k * gelem:(k + 1) * gelem],
                            out_offset=None,
                            in_=table,
                            in_offset=bass.IndirectOffsetOnAxis(
                                ap=idx[:, j:j + 1], axis=0),
                        )
                    Gfirst = Gb
                if not do_compute:
                    continue
                O = op.tile([P, NPT * C], FDT, tag="O")
                for k in range(NPT):
                    j = t * NPT + k
                    G = Gb[:, k * 4 * C:(k + 1) * 4 * C]
                    # first (4-asplit) chunks weighted on DVE, rest on ACT
                    nd = 4 - asplit
                    M = mp.tile([P, 4 * C], FDT, tag="M")
                    if nd > 0:
                        nc.vector.tensor_mul(
                            M[:, :nd * C].rearrange("p (a c) -> p a c", a=nd),
                            G[:, :nd * C].rearrange("p (a c) -> p a c", a=nd),
                            W3f[:, j, 0:nd].unsqueeze(2)
                            .to_broadcast([P, nd, C]),
                        )
                    for a in range(nd, 4):
                        nc.scalar.activation(
                            M[:, a * C:(a + 1) * C], G[:, a * C:(a + 1) * C],
                            mybir.ActivationFunctionType.Copy,
                            bias=0.0, scale=W3[:, j, a:a + 1])
                    S = sp.tile([P, 2 * C], FDT, tag="S")
                    nc.vector.tensor_add(S[:], M[:, :2 * C], M[:, 2 * C:])
                    nc.vector.tensor_add(O[:, k * C:(k + 1) * C],
                                         S[:, :C], S[:, C:])
                if do_store:
                    nc.sync.dma_start(out[t * P:(t + 1) * P, :], O[:])
    nc.compile()
    return nc


def _get_program():
    if "nc" not in _CACHED:
        _CACHED["nc"] = build_program(dt16=True)
    return _CACHED["nc"]


def _make_table2(feats, dtype=np.float32):
    """feats: [B,C,H,W] f32 -> list of B arrays [H*W, 2C] (channel-last,
    row y and y+1 concatenated)."""
    tables = []
    for b in range(B):
        bev = np.ascontiguousarray(feats[b].transpose(1, 2, 0)).astype(dtype)
        nxt = bev[np.minimum(np.arange(H) + 1, H - 1)]           # [H,W,C]
        t2 = np.concatenate([bev, nxt], axis=2)                  # [H,W,2C]
        tables.append(np.ascontiguousarray(t2.reshape(H * W, 2 * C)))
    return tables


def kernel(spatial_features_2d, rois, _want_results=False):
    feats = np.asarray(spatial_features_2d, dtype=np.float32)
    rois_np = np.asarray(rois, dtype=np.float32)
    assert feats.shape == (B, C, H, W) and rois_np.shape == (B, N, 7)

    nc = _get_program()
    tables = _make_table2(feats, np.float16)
    in_maps = []
    for core in range(NCORES):
        b, h = divmod(core, 2)
        in_maps.append({
            "table2": tables[b],
            "rois": np.ascontiguousarray(rois_np[b, h * NR:(h + 1) * NR]),
        })

    try:
        res = run_bass_kernel_spmd(
            nc, in_maps, list(range(NCORES)),
            trace=bool(int(os.environ.get("BEV_TRACE", "0"))),
        )
    except ModuleNotFoundError:
        # BASS_TRACE in the environment routes through the NTFF profile
        # hook (antenv.axon_hooks), which some containers lack. Degrade to
        # an untraced run instead of failing.
        os.environ["BASS_NEVER_TRACE"] = "1"
        try:
            res = run_bass_kernel_spmd(nc, in_maps, list(range(NCORES)),
                                       trace=False)
        finally:
            os.environ.pop("BASS_NEVER_TRACE", None)

    out = np.empty((B, N, NPT * C), dtype=np.float32)
    for core in range(NCORES):
        b, h = divmod(core, 2)
        out[b, h * NR:(h + 1) * NR] = res.results[core]["out"].astype(
            np.float32)
    if _want_results:
        return out, res
    return out

